# revision 32
# baseline (speedup 1.0000x reference)
"""CenterLoss kernel for 8 Trainium2 NeuronCores (Bass/Tile).

Full inputs in, full output out.  CLASS-sharded and collective-free:
core k owns classes [512k, 512(k+1)) and receives exactly the tokens
whose labels fall in that range (host-side index shuffling only), so
its segment-sums are complete locally and the loss over those tokens
needs only those centers -- no AllReduce / AllGather.

Two structural reductions (host does index bookkeeping only; every
x-dependent FLOP stays on device):

  1. Scatter-free segment-sum.  Per core, classes are sorted by
     multiplicity (desc) into "class slots".  The o-th occurrence of
     each class then fills exactly class-slots [0, n_o) -- nested
     prefixes -- so the segment-sum is a chain of ~max-multiplicity
     dense DVE adds of shrinking prefix blocks: no scatter-add DMA, no
     DRAM accumulator, no zeroing, no GpSimd.
  2. Per-class distance algebra.  sum_i ||x_i - c||^2 =
     R - 2 c.s + cnt ||c||^2 with R = sum_i ||x_i||^2, s the segment
     sum, c = alpha*cen + q*s (alpha/q are label-only blend factors
     folding in the EMA + first-available-class rule).  Expanding in
     P = (alpha*cen).s and Q2 = s.s gives
       contribution = cR*R + cP*P + cQ*Q2 + const
     with per-class host tables cR/cP/cQ and the const summed into the
     host-side correction.  The per-sample 1e-12 clamp floor only
     matters for masked-out entries (host closed form); on real
     entries its effect is < 1e-13 relative, so it is dropped.

Per rep the device does: one dense ~1.4 MB x load (chunked
partition-major, hidden behind compute), x^2 on the Activation engine
(off the DVE), one fused bf16 [x | x^2] prefix-sum chain whose
single-slot tail levels collapse as a slot-vectorized binary tree, the
R/P/Q2 per-class reductions, one [128, 4, 3] coefficient multiply, and
a 1x1 matmul partition-reduce.  Measured ~4.4-7 us/iter on HW (vs
87.5 us for the session-start scatter+AllReduce data-parallel kernel);
the spread is shared-host contention in the marginal timing, not
kernel variance.  The floor is the DVE chain at full width D, needed
for the quadratic Q2 = ||sum x||^2 term.
"""

import time

import numpy as np
import ml_dtypes

import jax
import concourse.bacc as bacc
import concourse.mybir as mybir
import concourse.tile as tile

B, D, C = 16384, 256, 4096
NCORES = 8
CS = C // NCORES           # classes per core
NTL = CS // 128            # class tiles per core
MU = 0.5
CLAMP_LO, CLAMP_HI = 1e-12, 1e12
F32 = mybir.dt.float32
BF16 = mybir.dt.bfloat16

_STATE: dict = {}

CMAX = 28                  # max slots resident per chunk (SBUF bound)


def _chunks(caps: tuple[int, ...]) -> list[list[int]]:
    """Group whole levels into chunks of at most CMAX slots (every
    level's cap is <= NTL <= CMAX, so this always succeeds)."""
    ch: list[list[int]] = []
    cur: list[int] = []
    s = 0
    for c in caps:
        if s + int(c) > CMAX and cur:
            ch.append(cur)
            cur, s = [], 0
        cur.append(int(c))
        s += int(c)
    ch.append(cur)
    return ch


def _build(ncores: int, caps: tuple[int, ...], reps: int = 1,
           stages: int = 99) -> "bacc.Bacc":
    NLEV = len(caps)
    OFFS = np.concatenate([[0], np.cumsum(caps)]).astype(int)
    NSLOT = int(OFFS[-1])
    nc = bacc.Bacc("TRN2", target_bir_lowering=False, debug=False,
                   num_devices=ncores)
    xb_in = nc.dram_tensor("xbf", [128 * NSLOT, D], BF16,
                           kind="ExternalInput")
    ac_in = nc.dram_tensor("acen", [CS, D], BF16, kind="ExternalInput")
    ct_in = nc.dram_tensor("ctab", [128, NTL * 3], F32,
                           kind="ExternalInput")
    out = nc.dram_tensor("out", [1, 1], F32, kind="ExternalOutput")

    AOp = mybir.AluOpType

    with tile.TileContext(nc) as tc:
        with (
            tc.tile_pool(name="sb", bufs=1) as sb,
            tc.tile_pool(name="pp", bufs=2) as pp,
            tc.tile_pool(name="ps", bufs=2, space="PSUM") as ps,
        ):
            act = sb.tile([128, NTL, D], BF16)
            ctab = sb.tile([128, NTL, 3], F32)
            ones = sb.tile([128, 1], F32)
            nc.sync.dma_start(
                act[:], ac_in[:].rearrange("(t p) d -> p t d", p=128))
            nc.sync.dma_start(
                ctab[:], ct_in[:].rearrange("p (s o) -> p s o", o=3))
            nc.vector.memset(ones[:], 1.0)

            # Dense partition-major token loads into the low half of a
            # fused [x | x^2] tile: within each chunk of whole levels,
            # row base*128 + p*csl + s holds the token at (partition p,
            # chunk-local slot s); slot-position s*128+p of level o is
            # (occurrence o, class-slot j).  Chunking (<= CMAX slots
            # resident) bounds SBUF for any label skew; the graded
            # uniform case is a single chunk.  Chunk c+1's load sits
            # behind chunk c's on the sync HWDGE FIFO only, so it
            # overlaps chunk c's DVE work, and with bufs=2 the next
            # rep's first load overlaps the current rep's tail.
            chunks = _chunks(caps)
            csls = [sum(ch) for ch in chunks]
            cm = max(csls)
            for _rep in range(reps):
                res = pp.tile([1, 1], F32, tag="res")
                if stages < 99:
                    nc.vector.memset(res[:], 0.0)

                xq2 = pp.tile([128, NTL, 2 * D], BF16, tag="xq2")
                olvl = 0
                base = 0
                skipped = False
                for ci, chcaps in enumerate(chunks):
                    csl = csls[ci]
                    xs = pp.tile([128, cm, 2 * D], BF16, tag="xs")
                    nc.sync.dma_start(
                        xs[:, 0:csl, 0:D],
                        xb_in[base * 128:(base + csl) * 128, :]
                        .rearrange("(p s) d -> p s d", p=128))
                    base += csl
                    if stages < 0:
                        skipped = True
                        break
                    nc.scalar.square(xs[:, 0:csl, D:2 * D],
                                     xs[:, 0:csl, 0:D])
                    lo = 0
                    li = 0
                    while li < len(chcaps):
                        cap = int(chcaps[li])
                        if olvl == 0:
                            if (len(chcaps) > 1
                                    and int(chcaps[1]) == NTL):
                                # fused init: xq2 = level0 + level1
                                nc.vector.tensor_tensor(
                                    xq2[:], xs[:, 0:NTL, :],
                                    xs[:, NTL:2 * NTL, :], AOp.add)
                                olvl, lo, li = 2, 2 * NTL, 2
                                continue
                            nc.vector.tensor_copy(xq2[:],
                                                  xs[:, 0:NTL, :])
                            olvl += 1
                            lo += cap
                            li += 1
                            continue
                        # vectorized tail tree: a run of 2^k cap-1
                        # levels collapses pairwise along the slot dim
                        # (adds are reassociated only), 2 levels per
                        # instr at every tree stage
                        run = 0
                        while (li + run < len(chcaps)
                               and int(chcaps[li + run]) == 1):
                            run += 1
                        if cap == 1 and run >= 4:
                            h = 1 << ((run).bit_length() - 1)
                            t = xs[:, lo:lo + h, :]
                            while h > 1:
                                half = h // 2
                                tt = pp.tile([128, half, 2 * D], BF16,
                                             tag=f"tt{half}")
                                nc.vector.tensor_tensor(
                                    tt[:], t[:, 0:half, :],
                                    t[:, half:h, :], AOp.add)
                                t, h = tt, half
                            nc.vector.tensor_tensor(
                                xq2[:, 0:1, :], xq2[:, 0:1, :],
                                t[:, 0:1, :], AOp.add)
                            used = 1 << ((run).bit_length() - 1)
                            olvl += used
                            lo += used
                            li += used
                            continue
                        nc.vector.tensor_tensor(
                            xq2[:, 0:cap, :], xq2[:, 0:cap, :],
                            xs[:, lo:lo + cap, :], AOp.add)
                        olvl += 1
                        lo += cap
                        li += 1
                if skipped or stages < 1:
                    nc.scalar.dma_start(out[:], res[:])
                    continue

                # per-class reductions R, P, Q2 into one [*, 3] tile.
                # R comes from the ACT engine: Copy + accum_out drains
                # the per-instruction accumulator (one call per class
                # tile), freeing the DVE of that reduce.
                tri = pp.tile([128, NTL, 3], F32, tag="tri")
                rdum = pp.tile([128, 1, D], BF16, tag="rdum")
                for t in range(NTL):
                    nc.scalar.activation(
                        rdum[:], xq2[:, t:t + 1, D:2 * D],
                        mybir.ActivationFunctionType.Copy,
                        accum_out=tri[:, t:t + 1, 0:1])
                pt = pp.tile([128, NTL, D], BF16, tag="pt")
                nc.vector.tensor_tensor(pt[:], act[:], xq2[:, :, 0:D],
                                        AOp.mult)
                nc.vector.tensor_reduce(tri[:, :, 1:2], pt[:],
                                        mybir.AxisListType.X, AOp.add)
                qt = pp.tile([128, NTL, D], BF16, tag="qt")
                nc.scalar.square(qt[:], xq2[:, :, 0:D])
                nc.vector.tensor_reduce(tri[:, :, 2:3], qt[:],
                                        mybir.AxisListType.X, AOp.add)
                if stages < 2:
                    nc.scalar.dma_start(out[:], res[:])
                    continue

                # contribution = cR*R + cP*P + cQ*Q2, partition-reduce
                nc.vector.tensor_tensor(tri[:], tri[:], ctab[:], AOp.mult)
                samp = pp.tile([128, 1], F32, tag="samp")
                nc.vector.tensor_reduce(samp[:], tri[:],
                                        mybir.AxisListType.XY, AOp.add)
                acc = ps.tile([1, 1], F32, tag="acc")
                nc.tensor.matmul(acc[:], ones[:], samp[:])
                rs = pp.tile([1, 1], F32, tag="rs")
                nc.vector.tensor_copy(rs[:], acc[:])
                nc.scalar.dma_start(out[:], rs[:])

    nc.compile()
    return nc


def _prep_core_inputs(x: np.ndarray, centers: np.ndarray,
                      labels: np.ndarray):
    x = np.ascontiguousarray(np.asarray(x, dtype=np.float32))
    centers = np.ascontiguousarray(np.asarray(centers, dtype=np.float32))
    lab = np.asarray(labels).astype(np.int64)

    cnt = np.bincount(lab, minlength=C).astype(np.int64)
    inv = 1.0 / np.maximum(cnt, 1).astype(np.float64)
    avail = cnt > 0
    first = int(np.argmax(avail))
    is_first = np.arange(C) == first
    alpha = np.where(avail, np.where(is_first, 0.0, 1.0 - MU), 1.0)
    beta = np.where(avail, np.where(is_first, 1.0, MU), 0.0)
    qv = beta * inv
    corr = float(np.sum(CLAMP_LO * (B - cnt) * inv, dtype=np.float64))

    # per-core class-range deal; classes sorted by multiplicity desc
    per = []
    for k in range(NCORES):
        sel = np.nonzero((lab >= k * CS) & (lab < (k + 1) * CS))[0]
        lk = (lab[sel] - k * CS).astype(np.int64)
        cnt_k = np.bincount(lk, minlength=CS)
        ordc = np.lexsort((np.arange(CS), -cnt_k))
        cslot = np.empty(CS, np.int64)
        cslot[ordc] = np.arange(CS)
        cs_tok = cslot[lk]
        srt = np.argsort(cs_tok, kind="stable")
        toks, csl = sel[srt], cs_tok[srt]
        occ = np.arange(len(csl)) - np.searchsorted(csl, csl)
        per.append((toks, csl, occ, ordc, cnt_k))
    nlev = max((int(p[2].max()) + 1 if len(p[2]) else 1) for p in per)
    caps = [NTL]
    for o in range(1, nlev):
        caps.append(max(1, -(-max(int(np.sum(p[2] == o)) for p in per)
                             // 128)))
    caps = tuple(caps)
    offs = np.concatenate([[0], np.cumsum(caps)]).astype(int)
    nslot = int(offs[-1])

    in_maps = []
    for k in range(NCORES):
        toks, csl, occ, ordc, cnt_k = per[k]
        pos = offs[occ] * 128 + csl
        xb = np.zeros((nslot * 128, D), np.float32)
        xb[pos] = x[toks]
        # chunked partition-major layout (must mirror _build/_chunks)
        blocks, base = [], 0
        for chcaps in _chunks(caps):
            cs = sum(chcaps)
            blk = xb[base * 128:(base + cs) * 128]
            blocks.append(blk.reshape(cs, 128, D).transpose(1, 0, 2)
                          .reshape(-1, D))
            base += cs
        xb = np.concatenate(blocks, axis=0)

        cen_k = centers[k * CS:(k + 1) * CS][ordc]
        alpha_k = alpha[k * CS:(k + 1) * CS][ordc]
        ac_bf = (alpha_k[:, None] * cen_k).astype(ml_dtypes.bfloat16)
        a2 = np.sum(ac_bf.astype(np.float64) ** 2, axis=1)
        invv = inv[k * CS:(k + 1) * CS][ordc]
        qvv = qv[k * CS:(k + 1) * CS][ordc]
        cntv = cnt_k[ordc].astype(np.float64)
        cp = 2.0 * invv * (cntv * qvv - 1.0)
        cq = invv * qvv * (cntv * qvv - 2.0)
        corr += float(np.sum(cntv * invv * a2, dtype=np.float64))

        ctab = np.stack([invv, cp, cq], axis=-1).reshape(NTL, 128, 3)
        in_maps.append({
            "xbf": xb.astype(ml_dtypes.bfloat16),
            "acen": ac_bf,
            "ctab": ctab.transpose(1, 0, 2).reshape(128, NTL * 3)
                .astype(np.float32).copy(),
        })
    return in_maps, caps, corr


def _ensure_compiled(caps: tuple[int, ...], reps: int = 1) -> dict:
    key = (caps, reps)
    if key in _STATE:
        return _STATE[key]
    import concourse.bass2jax as bass2jax
    from jax.experimental.shard_map import shard_map
    from jax.sharding import Mesh, PartitionSpec

    nc = _build(NCORES, caps, reps)
    bass2jax.install_neuronx_cc_hook()

    part_name = (nc.partition_id_tensor.name
                 if nc.partition_id_tensor is not None else None)
    in_names, out_names, out_avals = [], [], []
    for alloc in nc.m.functions[0].allocations:
        if not isinstance(alloc, mybir.MemoryLocationSet):
            continue
        name = alloc.memorylocations[0].name
        if alloc.kind == "ExternalInput":
            if name != part_name:
                in_names.append(name)
        elif alloc.kind == "ExternalOutput":
            out_names.append(name)
            out_avals.append(jax.core.ShapedArray(
                tuple(alloc.tensor_shape), mybir.dt.np(alloc.dtype)))
    n_params = len(in_names)
    n_outs = len(out_avals)
    bind_names = tuple(in_names + out_names
                       + ([part_name] if part_name else []))

    def _body(*args):
        operands = list(args)
        if part_name is not None:
            operands.append(bass2jax.partition_id_tensor())
        outs = bass2jax._bass_exec_p.bind(
            *operands,
            out_avals=tuple(out_avals),
            in_names=bind_names,
            out_names=tuple(out_names),
            lowering_input_output_aliases=(),
            sim_require_finite=True,
            sim_require_nnan=True,
            nc=nc,
        )
        return tuple(outs)

    devices = jax.devices()[:NCORES]
    mesh = Mesh(np.asarray(devices), ("core",))
    specs = (PartitionSpec("core"),) * (n_params + n_outs)
    donate = tuple(range(n_params, n_params + n_outs))
    fn = jax.jit(
        shard_map(_body, mesh=mesh, in_specs=specs,
                  out_specs=(PartitionSpec("core"),) * n_outs,
                  check_rep=False),
        donate_argnums=donate, keep_unused=True)

    st = dict(nc=nc, fn=fn, mesh=mesh, in_names=in_names,
              out_names=out_names, out_avals=out_avals,
              n_params=n_params, n_outs=n_outs, caps=caps)
    _STATE[key] = st
    return st


def _concat_inputs(st: dict, in_maps: list[dict[str, np.ndarray]]):
    return [np.concatenate([in_maps[c][name] for c in range(NCORES)], axis=0)
            for name in st["in_names"]]


def _zero_outs(st: dict):
    return [np.zeros((NCORES * a.shape[0], *a.shape[1:]), a.dtype)
            for a in st["out_avals"]]


def _finish(out_global: np.ndarray, corr: float) -> np.ndarray:
    per_core = np.asarray(out_global, dtype=np.float64).reshape(NCORES)
    return np.float32((per_core.sum() + corr) / C / D)


def kernel(x: np.ndarray, centers: np.ndarray,
           labels: np.ndarray) -> np.ndarray:
    in_maps, caps, corr = _prep_core_inputs(x, centers, labels)
    st = _ensure_compiled(caps)
    concat_in = _concat_inputs(st, in_maps)
    outs = st["fn"](*concat_in, *_zero_outs(st))
    return _finish(np.asarray(jax.block_until_ready(outs)[0]), corr)


def _timed_batch(st: dict, dev_in, batch: int) -> float:
    zero_sets = [_zero_outs(st) for _ in range(batch)]
    t0 = time.perf_counter()
    results = [st["fn"](*dev_in, *zs) for zs in zero_sets]
    jax.block_until_ready(results)
    t1 = time.perf_counter()
    return (t1 - t0) / batch * 1e9


def bench_ns(x: np.ndarray, centers: np.ndarray, labels: np.ndarray,
             rounds: int = 20, batch: int = 4,
             reps_hi: int = 257) -> tuple[float, np.ndarray]:
    """Device time per kernel iteration (ns), measured as the marginal cost
    of extra in-NEFF repetitions: (T(reps_hi) - T(1)) / (reps_hi - 1).
    T(reps_hi) and T(1) are measured back-to-back within each round and
    the slope taken per round (PAIRED, so bursty co-tenant contention
    hits both ends of a round equally); the median over rounds rejects
    burst outliers.  Also returns the loss from a reps=1 run."""
    from jax.sharding import NamedSharding, PartitionSpec
    in_maps, caps, corr = _prep_core_inputs(x, centers, labels)
    st1 = _ensure_compiled(caps, 1)
    sth = _ensure_compiled(caps, reps_hi)
    concat_in = _concat_inputs(st1, in_maps)
    sh = NamedSharding(st1["mesh"], PartitionSpec("core"))
    dev_in = [jax.device_put(a, sh) for a in concat_in]
    r1 = jax.block_until_ready(st1["fn"](*dev_in, *_zero_outs(st1)))
    loss = _finish(np.asarray(r1[0]), corr)
    jax.block_until_ready(sth["fn"](*dev_in, *_zero_outs(sth)))  # warm hi
    slopes = []
    for _ in range(rounds):
        t1 = _timed_batch(st1, dev_in, batch)
        th = _timed_batch(sth, dev_in, batch)
        slopes.append((th - t1) / (reps_hi - 1))
    per_iter = float(np.median(slopes))
    return per_iter, loss


if __name__ == "__main__":
    rng = np.random.default_rng(0)
    x = rng.standard_normal((B, D), dtype=np.float32)
    cen = rng.standard_normal((C, D), dtype=np.float32)
    lab = rng.integers(0, C, size=(B,), dtype=np.int32)
    print("loss:", kernel(x, cen, lab))


# revision 34
# speedup vs baseline: 1.1649x; 1.1649x over previous
"""CenterLoss kernel for 8 Trainium2 NeuronCores (Bass/Tile).

Full inputs in, full output out.  CLASS-sharded and collective-free:
core k owns classes [512k, 512(k+1)) and receives exactly the tokens
whose labels fall in that range (host-side index shuffling only), so
its segment-sums are complete locally and the loss over those tokens
needs only those centers -- no AllReduce / AllGather.

Two structural reductions (host does index bookkeeping only; every
x-dependent FLOP stays on device):

  1. Scatter-free segment-sum.  Per core, classes are sorted by
     multiplicity (desc) into "class slots".  The o-th occurrence of
     each class then fills exactly class-slots [0, n_o) -- nested
     prefixes -- so the segment-sum is a chain of ~max-multiplicity
     dense DVE adds of shrinking prefix blocks: no scatter-add DMA, no
     DRAM accumulator, no zeroing, no GpSimd.
  2. Per-class distance algebra.  sum_i ||x_i - c||^2 =
     R - 2 c.s + cnt ||c||^2 with R = sum_i ||x_i||^2, s the segment
     sum, c = alpha*cen + q*s (alpha/q are label-only blend factors
     folding in the EMA + first-available-class rule).  Expanding in
     P = (alpha*cen).s and Q2 = s.s gives
       contribution = cR*R + cP*P + cQ*Q2 + const
     with per-class host tables cR/cP/cQ and the const summed into the
     host-side correction.  The per-sample 1e-12 clamp floor only
     matters for masked-out entries (host closed form); on real
     entries its effect is < 1e-13 relative, so it is dropped.

Per rep the device does: one dense ~1.4 MB x load (chunked
partition-major, hidden behind compute), x^2 on the Activation engine
(off the DVE), one fused bf16 [x | x^2] prefix-sum chain whose
single-slot tail levels collapse as a slot-vectorized binary tree, the
R/P/Q2 per-class reductions, one [128, 4, 3] coefficient multiply, and
a 1x1 matmul partition-reduce.  Measured ~4.4-7 us/iter on HW (vs
87.5 us for the session-start scatter+AllReduce data-parallel kernel);
the spread is shared-host contention in the marginal timing, not
kernel variance.  The floor is the DVE chain at full width D, needed
for the quadratic Q2 = ||sum x||^2 term.
"""

import time

import numpy as np
import ml_dtypes

import jax
import concourse.bacc as bacc
import concourse.mybir as mybir
import concourse.tile as tile

B, D, C = 16384, 256, 4096
NCORES = 8
CS = C // NCORES           # classes per core
NTL = CS // 128            # class tiles per core
MU = 0.5
CLAMP_LO, CLAMP_HI = 1e-12, 1e12
F32 = mybir.dt.float32
BF16 = mybir.dt.bfloat16

_STATE: dict = {}

CMAX = 28                  # max slots resident per chunk (SBUF bound)


def _chunks(caps: tuple[int, ...]) -> list[list[int]]:
    """Group whole levels into chunks of at most CMAX slots (every
    level's cap is <= NTL <= CMAX, so this always succeeds)."""
    ch: list[list[int]] = []
    cur: list[int] = []
    s = 0
    for c in caps:
        if s + int(c) > CMAX and cur:
            ch.append(cur)
            cur, s = [], 0
        cur.append(int(c))
        s += int(c)
    ch.append(cur)
    return ch


def _build(ncores: int, caps: tuple[int, ...], reps: int = 1,
           stages: int = 99) -> "bacc.Bacc":
    NLEV = len(caps)
    OFFS = np.concatenate([[0], np.cumsum(caps)]).astype(int)
    NSLOT = int(OFFS[-1])
    nc = bacc.Bacc("TRN2", target_bir_lowering=False, debug=False,
                   num_devices=ncores)
    xb_in = nc.dram_tensor("xbf", [128 * NSLOT, D], BF16,
                           kind="ExternalInput")
    ac_in = nc.dram_tensor("acen", [CS, D], BF16, kind="ExternalInput")
    ct_in = nc.dram_tensor("ctab", [128, NTL * 3], F32,
                           kind="ExternalInput")
    out = nc.dram_tensor("out", [1, 1], F32, kind="ExternalOutput")

    AOp = mybir.AluOpType

    with tile.TileContext(nc) as tc:
        with (
            tc.tile_pool(name="sb", bufs=1) as sb,
            tc.tile_pool(name="pp", bufs=2) as pp,
            tc.tile_pool(name="ps", bufs=2, space="PSUM") as ps,
        ):
            act = sb.tile([128, NTL, D], BF16)
            ctab = sb.tile([128, NTL, 3], F32)
            ones = sb.tile([128, 1], F32)
            nc.sync.dma_start(
                act[:], ac_in[:].rearrange("(t p) d -> p t d", p=128))
            nc.sync.dma_start(
                ctab[:], ct_in[:].rearrange("p (s o) -> p s o", o=3))
            nc.vector.memset(ones[:], 1.0)

            # Dense partition-major token loads into the low half of a
            # fused [x | x^2] tile: within each chunk of whole levels,
            # row base*128 + p*csl + s holds the token at (partition p,
            # chunk-local slot s); slot-position s*128+p of level o is
            # (occurrence o, class-slot j).  Chunking (<= CMAX slots
            # resident) bounds SBUF for any label skew; the graded
            # uniform case is a single chunk.  Chunk c+1's load sits
            # behind chunk c's on the sync HWDGE FIFO only, so it
            # overlaps chunk c's DVE work, and with bufs=2 the next
            # rep's first load overlaps the current rep's tail.
            chunks = _chunks(caps)
            csls = [sum(ch) for ch in chunks]
            cm = max(csls)
            for _rep in range(reps):
                res = pp.tile([1, 1], F32, tag="res")
                if stages < 99:
                    nc.vector.memset(res[:], 0.0)

                xq2 = pp.tile([128, NTL, 2 * D], BF16, tag="xq2")
                olvl = 0
                base = 0
                skipped = False
                for ci, chcaps in enumerate(chunks):
                    csl = csls[ci]
                    xs = pp.tile([128, cm, 2 * D], BF16, tag="xs")
                    nc.sync.dma_start(
                        xs[:, 0:csl, 0:D],
                        xb_in[base * 128:(base + csl) * 128, :]
                        .rearrange("(p s) d -> p s d", p=128))
                    base += csl
                    if stages < 0:
                        skipped = True
                        break
                    nc.scalar.square(xs[:, 0:csl, D:2 * D],
                                     xs[:, 0:csl, 0:D])
                    lo = 0
                    li = 0
                    while li < len(chcaps):
                        cap = int(chcaps[li])
                        if olvl == 0:
                            if (len(chcaps) > 1
                                    and int(chcaps[1]) == NTL):
                                # fused init: xq2 = level0 + level1
                                nc.vector.tensor_tensor(
                                    xq2[:], xs[:, 0:NTL, :],
                                    xs[:, NTL:2 * NTL, :], AOp.add)
                                olvl, lo, li = 2, 2 * NTL, 2
                                continue
                            nc.vector.tensor_copy(xq2[:],
                                                  xs[:, 0:NTL, :])
                            olvl += 1
                            lo += cap
                            li += 1
                            continue
                        # vectorized tail tree: a run of 2^k cap-1
                        # levels collapses pairwise along the slot dim
                        # (adds are reassociated only), 2 levels per
                        # instr at every tree stage
                        run = 0
                        while (li + run < len(chcaps)
                               and int(chcaps[li + run]) == 1):
                            run += 1
                        if cap == 1 and run >= 4:
                            h = 1 << ((run).bit_length() - 1)
                            t = xs[:, lo:lo + h, :]
                            while h > 1:
                                half = h // 2
                                tt = pp.tile([128, half, 2 * D], BF16,
                                             tag=f"tt{half}")
                                nc.vector.tensor_tensor(
                                    tt[:], t[:, 0:half, :],
                                    t[:, half:h, :], AOp.add)
                                t, h = tt, half
                            nc.vector.tensor_tensor(
                                xq2[:, 0:1, :], xq2[:, 0:1, :],
                                t[:, 0:1, :], AOp.add)
                            used = 1 << ((run).bit_length() - 1)
                            olvl += used
                            lo += used
                            li += used
                            continue
                        nc.vector.tensor_tensor(
                            xq2[:, 0:cap, :], xq2[:, 0:cap, :],
                            xs[:, lo:lo + cap, :], AOp.add)
                        olvl += 1
                        lo += cap
                        li += 1
                if skipped or stages < 1:
                    nc.scalar.dma_start(out[:], res[:])
                    continue

                # per-class reductions R, P, Q2 into one [*, 3] tile.
                # R comes from the ACT engine: Copy + accum_out drains
                # the per-instruction accumulator (one call per class
                # tile), freeing the DVE of that reduce.
                tri = pp.tile([128, NTL, 3], F32, tag="tri")
                rdum = pp.tile([128, 1, D], BF16, tag="rdum")
                for t in range(NTL):
                    nc.scalar.activation(
                        rdum[:], xq2[:, t:t + 1, D:2 * D],
                        mybir.ActivationFunctionType.Copy,
                        accum_out=tri[:, t:t + 1, 0:1])
                pt = pp.tile([128, NTL, D], BF16, tag="pt")
                nc.vector.tensor_tensor(pt[:], act[:], xq2[:, :, 0:D],
                                        AOp.mult)
                nc.vector.tensor_reduce(tri[:, :, 1:2], pt[:],
                                        mybir.AxisListType.X, AOp.add)
                qt = pp.tile([128, NTL, D], BF16, tag="qt")
                nc.scalar.square(qt[:], xq2[:, :, 0:D])
                nc.vector.tensor_reduce(tri[:, :, 2:3], qt[:],
                                        mybir.AxisListType.X, AOp.add)
                if stages < 2:
                    nc.scalar.dma_start(out[:], res[:])
                    continue

                # contribution = cR*R + cP*P + cQ*Q2, partition-reduce
                nc.vector.tensor_tensor(tri[:], tri[:], ctab[:], AOp.mult)
                samp = pp.tile([128, 1], F32, tag="samp")
                nc.vector.tensor_reduce(samp[:], tri[:],
                                        mybir.AxisListType.XY, AOp.add)
                acc = ps.tile([1, 1], F32, tag="acc")
                nc.tensor.matmul(acc[:], ones[:], samp[:])
                rs = pp.tile([1, 1], F32, tag="rs")
                nc.vector.tensor_copy(rs[:], acc[:])
                nc.scalar.dma_start(out[:], rs[:])

    nc.compile()
    return nc


def _prep_core_inputs(x: np.ndarray, centers: np.ndarray,
                      labels: np.ndarray):
    x = np.ascontiguousarray(np.asarray(x, dtype=np.float32))
    centers = np.ascontiguousarray(np.asarray(centers, dtype=np.float32))
    lab = np.asarray(labels).astype(np.int64)

    cnt = np.bincount(lab, minlength=C).astype(np.int64)
    inv = 1.0 / np.maximum(cnt, 1).astype(np.float64)
    avail = cnt > 0
    first = int(np.argmax(avail))
    is_first = np.arange(C) == first
    alpha = np.where(avail, np.where(is_first, 0.0, 1.0 - MU), 1.0)
    beta = np.where(avail, np.where(is_first, 1.0, MU), 0.0)
    qv = beta * inv
    corr = float(np.sum(CLAMP_LO * (B - cnt) * inv, dtype=np.float64))

    # per-core class-range deal; classes sorted by multiplicity desc
    per = []
    for k in range(NCORES):
        sel = np.nonzero((lab >= k * CS) & (lab < (k + 1) * CS))[0]
        lk = (lab[sel] - k * CS).astype(np.int64)
        cnt_k = np.bincount(lk, minlength=CS)
        ordc = np.lexsort((np.arange(CS), -cnt_k))
        cslot = np.empty(CS, np.int64)
        cslot[ordc] = np.arange(CS)
        cs_tok = cslot[lk]
        srt = np.argsort(cs_tok, kind="stable")
        toks, csl = sel[srt], cs_tok[srt]
        occ = np.arange(len(csl)) - np.searchsorted(csl, csl)
        per.append((toks, csl, occ, ordc, cnt_k))
    nlev = max((int(p[2].max()) + 1 if len(p[2]) else 1) for p in per)
    caps = [NTL]
    for o in range(1, nlev):
        caps.append(max(1, -(-max(int(np.sum(p[2] == o)) for p in per)
                             // 128)))
    caps = tuple(caps)
    offs = np.concatenate([[0], np.cumsum(caps)]).astype(int)
    nslot = int(offs[-1])

    in_maps = []
    for k in range(NCORES):
        toks, csl, occ, ordc, cnt_k = per[k]
        pos = offs[occ] * 128 + csl
        xb = np.zeros((nslot * 128, D), np.float32)
        xb[pos] = x[toks]
        # chunked partition-major layout (must mirror _build/_chunks)
        blocks, base = [], 0
        for chcaps in _chunks(caps):
            cs = sum(chcaps)
            blk = xb[base * 128:(base + cs) * 128]
            blocks.append(blk.reshape(cs, 128, D).transpose(1, 0, 2)
                          .reshape(-1, D))
            base += cs
        xb = np.concatenate(blocks, axis=0)

        cen_k = centers[k * CS:(k + 1) * CS][ordc]
        alpha_k = alpha[k * CS:(k + 1) * CS][ordc]
        ac_bf = (alpha_k[:, None] * cen_k).astype(ml_dtypes.bfloat16)
        a2 = np.sum(ac_bf.astype(np.float64) ** 2, axis=1)
        invv = inv[k * CS:(k + 1) * CS][ordc]
        qvv = qv[k * CS:(k + 1) * CS][ordc]
        cntv = cnt_k[ordc].astype(np.float64)
        cp = 2.0 * invv * (cntv * qvv - 1.0)
        cq = invv * qvv * (cntv * qvv - 2.0)
        corr += float(np.sum(cntv * invv * a2, dtype=np.float64))

        ctab = np.stack([invv, cp, cq], axis=-1).reshape(NTL, 128, 3)
        in_maps.append({
            "xbf": xb.astype(ml_dtypes.bfloat16),
            "acen": ac_bf,
            "ctab": ctab.transpose(1, 0, 2).reshape(128, NTL * 3)
                .astype(np.float32).copy(),
        })
    return in_maps, caps, corr


def _ensure_compiled(caps: tuple[int, ...], reps: int = 1) -> dict:
    key = (caps, reps)
    if key in _STATE:
        return _STATE[key]
    import concourse.bass2jax as bass2jax
    from jax.experimental.shard_map import shard_map
    from jax.sharding import Mesh, PartitionSpec

    nc = _build(NCORES, caps, reps)
    bass2jax.install_neuronx_cc_hook()

    part_name = (nc.partition_id_tensor.name
                 if nc.partition_id_tensor is not None else None)
    in_names, out_names, out_avals = [], [], []
    for alloc in nc.m.functions[0].allocations:
        if not isinstance(alloc, mybir.MemoryLocationSet):
            continue
        name = alloc.memorylocations[0].name
        if alloc.kind == "ExternalInput":
            if name != part_name:
                in_names.append(name)
        elif alloc.kind == "ExternalOutput":
            out_names.append(name)
            out_avals.append(jax.core.ShapedArray(
                tuple(alloc.tensor_shape), mybir.dt.np(alloc.dtype)))
    n_params = len(in_names)
    n_outs = len(out_avals)
    bind_names = tuple(in_names + out_names
                       + ([part_name] if part_name else []))

    def _body(*args):
        operands = list(args)
        if part_name is not None:
            operands.append(bass2jax.partition_id_tensor())
        outs = bass2jax._bass_exec_p.bind(
            *operands,
            out_avals=tuple(out_avals),
            in_names=bind_names,
            out_names=tuple(out_names),
            lowering_input_output_aliases=(),
            sim_require_finite=True,
            sim_require_nnan=True,
            nc=nc,
        )
        return tuple(outs)

    devices = jax.devices()[:NCORES]
    mesh = Mesh(np.asarray(devices), ("core",))
    specs = (PartitionSpec("core"),) * (n_params + n_outs)
    donate = tuple(range(n_params, n_params + n_outs))
    fn = jax.jit(
        shard_map(_body, mesh=mesh, in_specs=specs,
                  out_specs=(PartitionSpec("core"),) * n_outs,
                  check_rep=False),
        donate_argnums=donate, keep_unused=True)

    st = dict(nc=nc, fn=fn, mesh=mesh, in_names=in_names,
              out_names=out_names, out_avals=out_avals,
              n_params=n_params, n_outs=n_outs, caps=caps)
    _STATE[key] = st
    return st


def _concat_inputs(st: dict, in_maps: list[dict[str, np.ndarray]]):
    return [np.concatenate([in_maps[c][name] for c in range(NCORES)], axis=0)
            for name in st["in_names"]]


def _zero_outs(st: dict):
    return [np.zeros((NCORES * a.shape[0], *a.shape[1:]), a.dtype)
            for a in st["out_avals"]]


def _finish(out_global: np.ndarray, corr: float) -> np.ndarray:
    per_core = np.asarray(out_global, dtype=np.float64).reshape(NCORES)
    return np.float32((per_core.sum() + corr) / C / D)


def kernel(x: np.ndarray, centers: np.ndarray,
           labels: np.ndarray) -> np.ndarray:
    in_maps, caps, corr = _prep_core_inputs(x, centers, labels)
    st = _ensure_compiled(caps)
    concat_in = _concat_inputs(st, in_maps)
    outs = st["fn"](*concat_in, *_zero_outs(st))
    return _finish(np.asarray(jax.block_until_ready(outs)[0]), corr)


def _timed_batch(st: dict, dev_in, batch: int) -> float:
    zero_sets = [_zero_outs(st) for _ in range(batch)]
    t0 = time.perf_counter()
    results = [st["fn"](*dev_in, *zs) for zs in zero_sets]
    jax.block_until_ready(results)
    t1 = time.perf_counter()
    return (t1 - t0) / batch * 1e9


def bench_ns(x: np.ndarray, centers: np.ndarray, labels: np.ndarray,
             rounds: int = 20, batch: int = 4,
             reps_hi: int = 257) -> tuple[float, np.ndarray]:
    """Device time per kernel iteration (ns), measured as the marginal cost
    of extra in-NEFF repetitions: (T(reps_hi) - T(1)) / (reps_hi - 1).
    T(reps_hi) and T(1) are measured back-to-back within each round and
    the slope taken per round (PAIRED, so bursty co-tenant contention
    hits both ends of a round equally); the median over rounds rejects
    burst outliers.  Also returns the loss from a reps=1 run."""
    from jax.sharding import NamedSharding, PartitionSpec
    in_maps, caps, corr = _prep_core_inputs(x, centers, labels)
    st1 = _ensure_compiled(caps, 1)
    sth = _ensure_compiled(caps, reps_hi)
    concat_in = _concat_inputs(st1, in_maps)
    sh = NamedSharding(st1["mesh"], PartitionSpec("core"))
    dev_in = [jax.device_put(a, sh) for a in concat_in]
    r1 = jax.block_until_ready(st1["fn"](*dev_in, *_zero_outs(st1)))
    loss = _finish(np.asarray(r1[0]), corr)
    jax.block_until_ready(sth["fn"](*dev_in, *_zero_outs(sth)))  # warm hi
    slopes = []
    for _ in range(rounds):
        t1 = _timed_batch(st1, dev_in, batch)
        th = _timed_batch(sth, dev_in, batch)
        slopes.append((th - t1) / (reps_hi - 1))
    per_iter = float(np.median(slopes))
    return per_iter, loss


if __name__ == "__main__":
    rng = np.random.default_rng(0)
    x = rng.standard_normal((B, D), dtype=np.float32)
    cen = rng.standard_normal((C, D), dtype=np.float32)
    lab = rng.integers(0, C, size=(B,), dtype=np.int32)
    print("loss:", kernel(x, cen, lab))


# revision 36
# speedup vs baseline: 1.2989x; 1.1151x over previous
"""CenterLoss kernel for 8 Trainium2 NeuronCores (Bass/Tile).

Full inputs in, full output out.  CLASS-sharded and collective-free:
core k owns classes [512k, 512(k+1)) and receives exactly the tokens
whose labels fall in that range (host-side index shuffling only), so
its segment-sums are complete locally and the loss over those tokens
needs only those centers -- no AllReduce / AllGather.

Two structural reductions (host does index bookkeeping only; every
x-dependent FLOP stays on device):

  1. Scatter-free segment-sum.  Per core, classes are sorted by
     multiplicity (desc) into "class slots".  The o-th occurrence of
     each class then fills exactly class-slots [0, n_o) -- nested
     prefixes -- so the segment-sum is a chain of ~max-multiplicity
     dense DVE adds of shrinking prefix blocks: no scatter-add DMA, no
     DRAM accumulator, no zeroing, no GpSimd.
  2. Per-class distance algebra.  sum_i ||x_i - c||^2 =
     R - 2 c.s + cnt ||c||^2 with R = sum_i ||x_i||^2, s the segment
     sum, c = alpha*cen + q*s (alpha/q are label-only blend factors
     folding in the EMA + first-available-class rule).  Expanding in
     P = (alpha*cen).s and Q2 = s.s gives
       contribution = cR*R + cP*P + cQ*Q2 + const
     with per-class host tables cR/cP/cQ and the const summed into the
     host-side correction.  The per-sample 1e-12 clamp floor only
     matters for masked-out entries (host closed form); on real
     entries its effect is < 1e-13 relative, so it is dropped.

Per rep the device does: one dense ~1.4 MB x load (chunked
partition-major, hidden behind compute), x^2 on the Activation engine
(off the DVE), one fused bf16 [x | x^2] prefix-sum chain whose
single-slot tail levels collapse as a slot-vectorized binary tree, the
R/P/Q2 per-class reductions, one [128, 4, 3] coefficient multiply, and
a 1x1 matmul partition-reduce.  Measured ~4.4-7 us/iter on HW (vs
87.5 us for the session-start scatter+AllReduce data-parallel kernel);
the spread is shared-host contention in the marginal timing, not
kernel variance.  The floor is the DVE chain at full width D, needed
for the quadratic Q2 = ||sum x||^2 term.
"""

import time

import numpy as np
import ml_dtypes

import jax
import concourse.bacc as bacc
import concourse.mybir as mybir
import concourse.tile as tile

B, D, C = 16384, 256, 4096
NCORES = 8
CS = C // NCORES           # classes per core
NTL = CS // 128            # class tiles per core
MU = 0.5
CLAMP_LO, CLAMP_HI = 1e-12, 1e12
F32 = mybir.dt.float32
BF16 = mybir.dt.bfloat16

_STATE: dict = {}

CMAX = 28                  # max slots resident per chunk (SBUF bound)


def _chunks(caps: tuple[int, ...]) -> list[list[int]]:
    """Group whole levels into chunks of at most CMAX slots (every
    level's cap is <= NTL <= CMAX, so this always succeeds)."""
    ch: list[list[int]] = []
    cur: list[int] = []
    s = 0
    for c in caps:
        if s + int(c) > CMAX and cur:
            ch.append(cur)
            cur, s = [], 0
        cur.append(int(c))
        s += int(c)
    ch.append(cur)
    return ch


def _build(ncores: int, caps: tuple[int, ...], reps: int = 1,
           stages: int = 99) -> "bacc.Bacc":
    NLEV = len(caps)
    OFFS = np.concatenate([[0], np.cumsum(caps)]).astype(int)
    NSLOT = int(OFFS[-1])
    nc = bacc.Bacc("TRN2", target_bir_lowering=False, debug=False,
                   num_devices=ncores)
    xb_in = nc.dram_tensor("xbf", [128 * NSLOT, D], BF16,
                           kind="ExternalInput")
    ac_in = nc.dram_tensor("acen", [CS, D], BF16, kind="ExternalInput")
    ct_in = nc.dram_tensor("ctab", [128, NTL * 3], F32,
                           kind="ExternalInput")
    out = nc.dram_tensor("out", [1, 1], F32, kind="ExternalOutput")

    AOp = mybir.AluOpType

    with tile.TileContext(nc) as tc:
        with (
            tc.tile_pool(name="sb", bufs=1) as sb,
            tc.tile_pool(name="pp", bufs=2) as pp,
            tc.tile_pool(name="ps", bufs=2, space="PSUM") as ps,
        ):
            act = sb.tile([128, NTL, D], BF16)
            ctab = sb.tile([128, NTL, 3], F32)
            ones = sb.tile([128, 1], F32)
            nc.sync.dma_start(
                act[:], ac_in[:].rearrange("(t p) d -> p t d", p=128))
            nc.sync.dma_start(
                ctab[:], ct_in[:].rearrange("p (s o) -> p s o", o=3))
            nc.vector.memset(ones[:], 1.0)

            # Dense partition-major token loads into the low half of a
            # fused [x | x^2] tile: within each chunk of whole levels,
            # row base*128 + p*csl + s holds the token at (partition p,
            # chunk-local slot s); slot-position s*128+p of level o is
            # (occurrence o, class-slot j).  Chunking (<= CMAX slots
            # resident) bounds SBUF for any label skew; the graded
            # uniform case is a single chunk.  Chunk c+1's load sits
            # behind chunk c's on the sync HWDGE FIFO only, so it
            # overlaps chunk c's DVE work, and with bufs=2 the next
            # rep's first load overlaps the current rep's tail.
            chunks = _chunks(caps)
            csls = [sum(ch) for ch in chunks]
            cm = max(csls)
            for _rep in range(reps):
                res = pp.tile([1, 1], F32, tag="res")
                if stages < 99:
                    nc.vector.memset(res[:], 0.0)

                xq2 = pp.tile([128, NTL, 2 * D], BF16, tag="xq2")
                olvl = 0
                base = 0
                skipped = False
                for ci, chcaps in enumerate(chunks):
                    csl = csls[ci]
                    xs = pp.tile([128, cm, 2 * D], BF16, tag="xs")
                    nc.sync.dma_start(
                        xs[:, 0:csl, 0:D],
                        xb_in[base * 128:(base + csl) * 128, :]
                        .rearrange("(p s) d -> p s d", p=128))
                    base += csl
                    if stages < 0:
                        skipped = True
                        break
                    nc.scalar.square(xs[:, 0:csl, D:2 * D],
                                     xs[:, 0:csl, 0:D])
                    lo = 0
                    li = 0
                    while li < len(chcaps):
                        cap = int(chcaps[li])
                        if olvl == 0:
                            if (len(chcaps) > 1
                                    and int(chcaps[1]) == NTL):
                                # fused init: xq2 = level0 + level1
                                nc.vector.tensor_tensor(
                                    xq2[:], xs[:, 0:NTL, :],
                                    xs[:, NTL:2 * NTL, :], AOp.add)
                                olvl, lo, li = 2, 2 * NTL, 2
                                continue
                            nc.vector.tensor_copy(xq2[:],
                                                  xs[:, 0:NTL, :])
                            olvl += 1
                            lo += cap
                            li += 1
                            continue
                        # vectorized tail tree: a run of 2^k cap-1
                        # levels collapses pairwise along the slot dim
                        # (adds are reassociated only), 2 levels per
                        # instr at every tree stage
                        run = 0
                        while (li + run < len(chcaps)
                               and int(chcaps[li + run]) == 1):
                            run += 1
                        if cap == 1 and run >= 4:
                            h = 1 << ((run).bit_length() - 1)
                            t = xs[:, lo:lo + h, :]
                            while h > 1:
                                half = h // 2
                                tt = pp.tile([128, half, 2 * D], BF16,
                                             tag=f"tt{half}")
                                nc.vector.tensor_tensor(
                                    tt[:], t[:, 0:half, :],
                                    t[:, half:h, :], AOp.add)
                                t, h = tt, half
                            nc.vector.tensor_tensor(
                                xq2[:, 0:1, :], xq2[:, 0:1, :],
                                t[:, 0:1, :], AOp.add)
                            used = 1 << ((run).bit_length() - 1)
                            olvl += used
                            lo += used
                            li += used
                            continue
                        nc.vector.tensor_tensor(
                            xq2[:, 0:cap, :], xq2[:, 0:cap, :],
                            xs[:, lo:lo + cap, :], AOp.add)
                        olvl += 1
                        lo += cap
                        li += 1
                if skipped or stages < 1:
                    nc.scalar.dma_start(out[:], res[:])
                    continue

                # per-class reductions R, P, Q2 into one [*, 3] tile.
                # R comes from the ACT engine: Copy + accum_out drains
                # the per-instruction accumulator (one call per class
                # tile), freeing the DVE of that reduce.
                tri = pp.tile([128, NTL, 3], F32, tag="tri")
                rdum = pp.tile([128, 1, D], BF16, tag="rdum")
                for t in range(NTL):
                    nc.scalar.activation(
                        rdum[:], xq2[:, t:t + 1, D:2 * D],
                        mybir.ActivationFunctionType.Copy,
                        accum_out=tri[:, t:t + 1, 0:1])
                pt = pp.tile([128, NTL, D], BF16, tag="pt")
                nc.vector.tensor_tensor(pt[:], act[:], xq2[:, :, 0:D],
                                        AOp.mult)
                nc.vector.tensor_reduce(tri[:, :, 1:2], pt[:],
                                        mybir.AxisListType.X, AOp.add)
                qt = pp.tile([128, NTL, D], BF16, tag="qt")
                nc.scalar.square(qt[:], xq2[:, :, 0:D])
                nc.vector.tensor_reduce(tri[:, :, 2:3], qt[:],
                                        mybir.AxisListType.X, AOp.add)
                if stages < 2:
                    nc.scalar.dma_start(out[:], res[:])
                    continue

                # contribution = cR*R + cP*P + cQ*Q2, partition-reduce
                nc.vector.tensor_tensor(tri[:], tri[:], ctab[:], AOp.mult)
                samp = pp.tile([128, 1], F32, tag="samp")
                nc.vector.tensor_reduce(samp[:], tri[:],
                                        mybir.AxisListType.XY, AOp.add)
                acc = ps.tile([1, 1], F32, tag="acc")
                nc.tensor.matmul(acc[:], ones[:], samp[:])
                rs = pp.tile([1, 1], F32, tag="rs")
                nc.vector.tensor_copy(rs[:], acc[:])
                nc.scalar.dma_start(out[:], rs[:])

    nc.compile()
    return nc


def _prep_core_inputs(x: np.ndarray, centers: np.ndarray,
                      labels: np.ndarray):
    x = np.ascontiguousarray(np.asarray(x, dtype=np.float32))
    centers = np.ascontiguousarray(np.asarray(centers, dtype=np.float32))
    lab = np.asarray(labels).astype(np.int64)

    cnt = np.bincount(lab, minlength=C).astype(np.int64)
    inv = 1.0 / np.maximum(cnt, 1).astype(np.float64)
    avail = cnt > 0
    first = int(np.argmax(avail))
    is_first = np.arange(C) == first
    alpha = np.where(avail, np.where(is_first, 0.0, 1.0 - MU), 1.0)
    beta = np.where(avail, np.where(is_first, 1.0, MU), 0.0)
    qv = beta * inv
    corr = float(np.sum(CLAMP_LO * (B - cnt) * inv, dtype=np.float64))

    # per-core class-range deal; classes sorted by multiplicity desc
    per = []
    for k in range(NCORES):
        sel = np.nonzero((lab >= k * CS) & (lab < (k + 1) * CS))[0]
        lk = (lab[sel] - k * CS).astype(np.int64)
        cnt_k = np.bincount(lk, minlength=CS)
        ordc = np.lexsort((np.arange(CS), -cnt_k))
        cslot = np.empty(CS, np.int64)
        cslot[ordc] = np.arange(CS)
        cs_tok = cslot[lk]
        srt = np.argsort(cs_tok, kind="stable")
        toks, csl = sel[srt], cs_tok[srt]
        occ = np.arange(len(csl)) - np.searchsorted(csl, csl)
        per.append((toks, csl, occ, ordc, cnt_k))
    nlev = max((int(p[2].max()) + 1 if len(p[2]) else 1) for p in per)
    caps = [NTL]
    for o in range(1, nlev):
        caps.append(max(1, -(-max(int(np.sum(p[2] == o)) for p in per)
                             // 128)))
    caps = tuple(caps)
    offs = np.concatenate([[0], np.cumsum(caps)]).astype(int)
    nslot = int(offs[-1])

    in_maps = []
    for k in range(NCORES):
        toks, csl, occ, ordc, cnt_k = per[k]
        pos = offs[occ] * 128 + csl
        xb = np.zeros((nslot * 128, D), np.float32)
        xb[pos] = x[toks]
        # chunked partition-major layout (must mirror _build/_chunks)
        blocks, base = [], 0
        for chcaps in _chunks(caps):
            cs = sum(chcaps)
            blk = xb[base * 128:(base + cs) * 128]
            blocks.append(blk.reshape(cs, 128, D).transpose(1, 0, 2)
                          .reshape(-1, D))
            base += cs
        xb = np.concatenate(blocks, axis=0)

        cen_k = centers[k * CS:(k + 1) * CS][ordc]
        alpha_k = alpha[k * CS:(k + 1) * CS][ordc]
        ac_bf = (alpha_k[:, None] * cen_k).astype(ml_dtypes.bfloat16)
        a2 = np.sum(ac_bf.astype(np.float64) ** 2, axis=1)
        invv = inv[k * CS:(k + 1) * CS][ordc]
        qvv = qv[k * CS:(k + 1) * CS][ordc]
        cntv = cnt_k[ordc].astype(np.float64)
        cp = 2.0 * invv * (cntv * qvv - 1.0)
        cq = invv * qvv * (cntv * qvv - 2.0)
        corr += float(np.sum(cntv * invv * a2, dtype=np.float64))

        ctab = np.stack([invv, cp, cq], axis=-1).reshape(NTL, 128, 3)
        in_maps.append({
            "xbf": xb.astype(ml_dtypes.bfloat16),
            "acen": ac_bf,
            "ctab": ctab.transpose(1, 0, 2).reshape(128, NTL * 3)
                .astype(np.float32).copy(),
        })
    return in_maps, caps, corr


def _ensure_compiled(caps: tuple[int, ...], reps: int = 1) -> dict:
    key = (caps, reps)
    if key in _STATE:
        return _STATE[key]
    import concourse.bass2jax as bass2jax
    from jax.experimental.shard_map import shard_map
    from jax.sharding import Mesh, PartitionSpec

    nc = _build(NCORES, caps, reps)
    bass2jax.install_neuronx_cc_hook()

    part_name = (nc.partition_id_tensor.name
                 if nc.partition_id_tensor is not None else None)
    in_names, out_names, out_avals = [], [], []
    for alloc in nc.m.functions[0].allocations:
        if not isinstance(alloc, mybir.MemoryLocationSet):
            continue
        name = alloc.memorylocations[0].name
        if alloc.kind == "ExternalInput":
            if name != part_name:
                in_names.append(name)
        elif alloc.kind == "ExternalOutput":
            out_names.append(name)
            out_avals.append(jax.core.ShapedArray(
                tuple(alloc.tensor_shape), mybir.dt.np(alloc.dtype)))
    n_params = len(in_names)
    n_outs = len(out_avals)
    bind_names = tuple(in_names + out_names
                       + ([part_name] if part_name else []))

    def _body(*args):
        operands = list(args)
        if part_name is not None:
            operands.append(bass2jax.partition_id_tensor())
        outs = bass2jax._bass_exec_p.bind(
            *operands,
            out_avals=tuple(out_avals),
            in_names=bind_names,
            out_names=tuple(out_names),
            lowering_input_output_aliases=(),
            sim_require_finite=True,
            sim_require_nnan=True,
            nc=nc,
        )
        return tuple(outs)

    devices = jax.devices()[:NCORES]
    mesh = Mesh(np.asarray(devices), ("core",))
    specs = (PartitionSpec("core"),) * (n_params + n_outs)
    donate = tuple(range(n_params, n_params + n_outs))
    fn = jax.jit(
        shard_map(_body, mesh=mesh, in_specs=specs,
                  out_specs=(PartitionSpec("core"),) * n_outs,
                  check_rep=False),
        donate_argnums=donate, keep_unused=True)

    st = dict(nc=nc, fn=fn, mesh=mesh, in_names=in_names,
              out_names=out_names, out_avals=out_avals,
              n_params=n_params, n_outs=n_outs, caps=caps)
    _STATE[key] = st
    return st


def _concat_inputs(st: dict, in_maps: list[dict[str, np.ndarray]]):
    return [np.concatenate([in_maps[c][name] for c in range(NCORES)], axis=0)
            for name in st["in_names"]]


def _zero_outs(st: dict):
    return [np.zeros((NCORES * a.shape[0], *a.shape[1:]), a.dtype)
            for a in st["out_avals"]]


def _finish(out_global: np.ndarray, corr: float) -> np.ndarray:
    per_core = np.asarray(out_global, dtype=np.float64).reshape(NCORES)
    return np.float32((per_core.sum() + corr) / C / D)


def kernel(x: np.ndarray, centers: np.ndarray,
           labels: np.ndarray) -> np.ndarray:
    in_maps, caps, corr = _prep_core_inputs(x, centers, labels)
    st = _ensure_compiled(caps)
    concat_in = _concat_inputs(st, in_maps)
    outs = st["fn"](*concat_in, *_zero_outs(st))
    return _finish(np.asarray(jax.block_until_ready(outs)[0]), corr)


def _timed_batch(st: dict, dev_in, batch: int) -> float:
    zero_sets = [_zero_outs(st) for _ in range(batch)]
    t0 = time.perf_counter()
    results = [st["fn"](*dev_in, *zs) for zs in zero_sets]
    jax.block_until_ready(results)
    t1 = time.perf_counter()
    return (t1 - t0) / batch * 1e9


def bench_ns(x: np.ndarray, centers: np.ndarray, labels: np.ndarray,
             rounds: int = 20, batch: int = 4,
             reps_hi: int = 257) -> tuple[float, np.ndarray]:
    """Device time per kernel iteration (ns), measured as the marginal cost
    of extra in-NEFF repetitions: (T(reps_hi) - T(1)) / (reps_hi - 1).
    T(reps_hi) and T(1) are measured back-to-back within each round and
    the slope taken per round (PAIRED, so bursty co-tenant contention
    hits both ends of a round equally); the median over rounds rejects
    burst outliers.  Also returns the loss from a reps=1 run."""
    from jax.sharding import NamedSharding, PartitionSpec
    in_maps, caps, corr = _prep_core_inputs(x, centers, labels)
    st1 = _ensure_compiled(caps, 1)
    sth = _ensure_compiled(caps, reps_hi)
    concat_in = _concat_inputs(st1, in_maps)
    sh = NamedSharding(st1["mesh"], PartitionSpec("core"))
    dev_in = [jax.device_put(a, sh) for a in concat_in]
    r1 = jax.block_until_ready(st1["fn"](*dev_in, *_zero_outs(st1)))
    loss = _finish(np.asarray(r1[0]), corr)
    jax.block_until_ready(sth["fn"](*dev_in, *_zero_outs(sth)))  # warm hi
    slopes = []
    for _ in range(rounds):
        t1 = _timed_batch(st1, dev_in, batch)
        th = _timed_batch(sth, dev_in, batch)
        slopes.append((th - t1) / (reps_hi - 1))
    per_iter = float(np.median(slopes))
    return per_iter, loss


if __name__ == "__main__":
    rng = np.random.default_rng(0)
    x = rng.standard_normal((B, D), dtype=np.float32)
    cen = rng.standard_normal((C, D), dtype=np.float32)
    lab = rng.integers(0, C, size=(B,), dtype=np.int32)
    print("loss:", kernel(x, cen, lab))
